# revision 3
# baseline (speedup 1.0000x reference)
"""ConvNetWordEncoder Trainium2 kernel.

Computes, for a batch of words (each a sequence of L=16 character ids):
  x = emb_table[words]                          # [L, N, D] character embeddings
  y = conv1d(x, conv_w, pad=1) + b              # [N, D, L]
  out = max_t relu(y)                           # [N, D]

Key algebraic rewrite: the embedding lookup is linear (x = E^T @ onehot), so
the conv collapses into per-tap fused tables M_k = E @ W_k^T of shape
[128 letters, 300 channels]:
  z_t[word, :] = sum_k M_k^T @ onehot(letter at t+k-1)
This shrinks the contraction from 900 (=D*K) to 128 (letters) per tap and
turns the embedding gather into the matmul itself.  relu/bias commute with
the temporal max, so bias+relu are applied once after the max.

Per core (data-parallel over N: 2048 words/core, 16 groups of 128 words):
  - PE broadcast-matmul (ones[1,128] stationary) replicates the letter ids
    across 128 partitions into PSUM.
  - ACT builds the exact one-hot via relu(1 - (id - p)^2) in two ops.
  - PE conv: for each s, onehot_s is the stationary operand [128l, 128w],
    the three fused tables stream as moving operands [128l, 300c],
    accumulating z_{s-1}, z_s, z_{s+1} in PSUM (float32r, 1 cycle/row).
  - DVE folds each completed z_t into a running max in SBUF.
  - DVE adds the (pre-broadcast) bias row, ACT applies relu, DMA out.
Output orientation is [words, channels] directly - no transposes anywhere.
"""

import numpy as np
from contextlib import ExitStack

import concourse.bass as bass
import concourse.tile as tile
from concourse import bacc, mybir
from concourse.bass_utils import run_bass_kernel_spmd

HIDDEN = 300
NLET = 128
KSIZE = 3
L = 16
NWORDS = 16384
NCORES = 8
NW = NWORDS // NCORES          # 2048 words per core
GROUPS = NW // 128             # 16 groups of 128 words
FP32 = mybir.dt.float32
FP32R = mybir.dt.float32r

_cache = {}


def _build(iters=1):
    if iters in _cache:
        return _cache[iters]
    nc = bacc.Bacc("TRN2", target_bir_lowering=False, debug=False,
                   num_devices=NCORES)

    wordsg_d = nc.dram_tensor("wordsg", [GROUPS, L * 128], FP32R,
                              kind="ExternalInput")
    wfuse_d = nc.dram_tensor("wfuse", [KSIZE, NLET, HIDDEN], FP32R,
                             kind="ExternalInput")
    biasrow_d = nc.dram_tensor("biasrow", [1, HIDDEN], FP32R,
                               kind="ExternalInput")
    ones_d = nc.dram_tensor("ones", [1, 128], FP32R, kind="ExternalInput")
    iotaneg_d = nc.dram_tensor("iotaneg", [128, 1], FP32,
                               kind="ExternalInput")
    out_d = nc.dram_tensor("out", [NW, HIDDEN], FP32, kind="ExternalOutput")

    Sq = mybir.ActivationFunctionType.Square
    Relu = mybir.ActivationFunctionType.Relu
    Max = mybir.AluOpType.max
    Add = mybir.AluOpType.add

    with tile.TileContext(nc) as tc, ExitStack() as ctx:
        const = ctx.enter_context(tc.tile_pool(name="const", bufs=1))
        wpool = ctx.enter_context(tc.tile_pool(name="wpool", bufs=2))
        ohpool = ctx.enter_context(tc.tile_pool(name="oh", bufs=2))
        d2pool = ctx.enter_context(tc.tile_pool(name="d2", bufs=2))
        rmpool = ctx.enter_context(tc.tile_pool(name="rm", bufs=2))
        outpool = ctx.enter_context(tc.tile_pool(name="outp", bufs=3))
        pb = ctx.enter_context(tc.tile_pool(name="pb", bufs=2, space="PSUM"))
        pz = ctx.enter_context(tc.tile_pool(name="pz", bufs=5, space="PSUM"))
        pbias = ctx.enter_context(tc.tile_pool(name="pbias", bufs=1,
                                               space="PSUM"))

        wfuse = const.tile([NLET, KSIZE, HIDDEN], FP32R)
        for k in range(KSIZE):
            nc.sync.dma_start(wfuse[:, k, :], wfuse_d.ap()[k])
        ones_t = const.tile([1, 128], FP32R)
        nc.sync.dma_start(ones_t[:], ones_d.ap()[:])
        iota_t = const.tile([128, 1], FP32)
        nc.sync.dma_start(iota_t[:], iotaneg_d.ap()[:])
        brow_t = const.tile([1, HIDDEN], FP32R)
        nc.sync.dma_start(brow_t[:], biasrow_d.ap()[:])

        # bias broadcast to [128 words, 300 ch] once (K=1 rank-1 matmul)
        bb_ps = pbias.tile([128, HIDDEN], FP32)
        nc.tensor.matmul(bb_ps[:], ones_t[:],
                         brow_t[:], start=True, stop=True)
        bias_bc = const.tile([128, HIDDEN], FP32)
        nc.scalar.copy(bias_bc[:], bb_ps[:])

        for _ in range(iters):
            for g in range(GROUPS):
                wtile = wpool.tile([1, L * 128], FP32R, tag="wtile")
                nc.sync.dma_start(wtile[:], wordsg_d.ap()[g:g + 1, :])
                ohblk = ohpool.tile([128, L * 128], FP32R, tag="ohblk")
                for q in range(4):
                    bps = pb.tile([128, 512], FP32, tag="bps")
                    nc.tensor.matmul(
                        bps[:], ones_t[:],
                        wtile[0:1, q * 512:(q + 1) * 512],
                        start=True, stop=True)
                    d2 = d2pool.tile([128, 512], FP32, tag="d2")
                    nc.scalar.activation(d2[:], bps[:], Sq, bias=iota_t[:],
                                         scale=1.0)
                    nc.scalar.activation(ohblk[:, q * 512:(q + 1) * 512],
                                         d2[:], Relu, bias=1.0, scale=-1.0)

                z = [None] * L
                runmax = rmpool.tile([128, HIDDEN], FP32, tag="runmax")
                for s in range(L):
                    oh_s = ohblk[:, s * 128:(s + 1) * 128]
                    if s == 0:
                        z[0] = pz.tile([128, HIDDEN], FP32, tag="z", name="z0")
                    if s + 1 < L:
                        z[s + 1] = pz.tile([128, HIDDEN], FP32, tag="z", name="zn")
                        nc.tensor.matmul(z[s + 1][:], oh_s,
                                         wfuse[:, 0, :],
                                         start=True, stop=False)
                    nc.tensor.matmul(z[s][:], oh_s,
                                     wfuse[:, 1, :],
                                     start=(s == 0), stop=(s == L - 1))
                    if s >= 1:
                        nc.tensor.matmul(z[s - 1][:], oh_s,
                                         wfuse[:, 2, :],
                                         start=False, stop=True)
                        if s == 1:
                            nc.vector.tensor_copy(runmax[:], z[0][:])
                        else:
                            nc.vector.tensor_tensor(runmax[:], z[s - 1][:],
                                                    runmax[:], Max)
                nc.vector.tensor_tensor(runmax[:], z[L - 1][:], runmax[:], Max)

                badd = outpool.tile([128, HIDDEN], FP32, tag="badd")
                nc.vector.tensor_tensor(badd[:], runmax[:], bias_bc[:], Add)
                outt = outpool.tile([128, HIDDEN], FP32, tag="outt")
                nc.scalar.activation(outt[:], badd[:], Relu)
                nc.sync.dma_start(out_d.ap()[g * 128:(g + 1) * 128, :],
                                  outt[:])

    nc.compile()
    _cache[iters] = nc
    return nc


def _prep_inputs(words_batch, emb_table, conv_w, conv_b):
    emb = np.asarray(emb_table, dtype=np.float32)
    w = np.asarray(conv_w, dtype=np.float32)
    b = np.asarray(conv_b, dtype=np.float32)
    words = np.asarray(words_batch)

    wfuse = np.stack([emb @ w[:, :, k].T for k in range(KSIZE)], axis=0)
    wfuse = np.ascontiguousarray(wfuse, dtype=np.float32)
    biasrow = np.ascontiguousarray(b.reshape(1, HIDDEN), dtype=np.float32)
    ones = np.ones((1, 128), np.float32)
    iotaneg = (-np.arange(128, dtype=np.float32)).reshape(128, 1)

    in_maps = []
    for c in range(NCORES):
        wc = words[:, c * NW:(c + 1) * NW].astype(np.float32)   # [16, 2048]
        wg = np.ascontiguousarray(
            wc.reshape(L, GROUPS, 128).transpose(1, 0, 2)
        ).reshape(GROUPS, L * 128)
        in_maps.append({
            "wordsg": wg, "wfuse": wfuse, "biasrow": biasrow,
            "ones": ones, "iotaneg": iotaneg,
        })
    return in_maps


def _run(in_maps, iters=1):
    nc = _build(iters)
    return run_bass_kernel_spmd(nc, in_maps, list(range(NCORES)),
                                trace=False)


def kernel(words_batch, emb_table, conv_w, conv_b):
    in_maps = _prep_inputs(words_batch, emb_table, conv_w, conv_b)
    res = _run(in_maps, iters=1)
    out = np.concatenate([res.results[c]["out"] for c in range(NCORES)],
                         axis=0)
    return out
